# revision 38
# baseline (speedup 1.0000x reference)
"""Trainium2 Bass kernel for nn_BSI_71597104824914.

Model (per batch b, node i): single-step LSTM (zero init state) over sliding
windows of x, then a per-node FC producing contrib[b,tau,j,i]; outputs
  G[b,j,i]    = mean_tau contrib / x[b,i,L+tau]
  X_hat[b,tau,i] = sum_j contrib + 1e-6

Sharding: nodes are split across the 8 NeuronCores (16 nodes/core), each core
processes all 8 batch elements for its nodes. Per (node):
  mm1 (PE):   gates[g, b, tau] = sum_l W_ih[n,g,l] * x[b,n,tau+l]   (K=32)
  ACT:        sigmoid/tanh batched over (b, tau), bias via ACT bias port
  DVE:        c = sig_i*tanh_g; hsc = tanh(c)*sig_o * 1/(224*x)  (bf16, 2x)
  mm2 (PE):   contrib_scaled[j, b, tau] = WfcT_n.T @ hsc          (K=128)
  X (PE):     transposed matvec: lhsT=hsc chunk -> X on 112 partitions
  DVE:        G col = reduce_tau(contrib_scaled); tiny X psum->sbuf copy
Host folds: bias terms of G/X, un-scaling of X, gather/transpose.
"""

import numpy as np
import ml_dtypes

import concourse.bass as bass
import concourse.bacc as bacc
import concourse.tile as tile
from concourse.tile import add_dep_helper
from concourse import mybir
from concourse.bass_utils import run_bass_kernel_spmd

BF16 = ml_dtypes.bfloat16

B, N, T, L, H = 8, 128, 256, 32, 128
TAU = T - L  # 224
NCORES = 8
NN = N // NCORES  # 16 nodes per core
NG = NN // 4  # node groups of 4 (row-tile positions)
CHUNKS = (0, 2, 3)  # pytorch gate order i,f,g,o -> we need i,g,o (f unused)

_cache = {}


def build_nc():
    nc = bacc.Bacc(None, target_bir_lowering=False)
    f32 = mybir.dt.float32
    bf16 = mybir.dt.bfloat16
    AF = mybir.ActivationFunctionType

    # x is node-major [NN, B, T] flattened, padded by 32 so the shifted
    # window reads of the last node stay in bounds.
    x_h = nc.dram_tensor("x", [NN * B * T + L], bf16, kind="ExternalInput")
    w1_h = nc.dram_tensor("w1", [NG, 128, 3, H], bf16, kind="ExternalInput")
    wfc_h = nc.dram_tensor("wfc", [NN, H, N], bf16, kind="ExternalInput")
    wsum_h = nc.dram_tensor("wsum", [H, NN], bf16, kind="ExternalInput")
    bias_h = nc.dram_tensor("bias", [H, NN, 3], f32, kind="ExternalInput")
    invx_h = nc.dram_tensor("invx", [NN, B, TAU], bf16, kind="ExternalInput")
    g_h = nc.dram_tensor("gout", [N, B, NN], f32, kind="ExternalOutput")
    xo_h = nc.dram_tensor("xout", [NN, B, TAU], f32, kind="ExternalOutput")

    with tile.TileContext(nc) as tc:
        with (
            tc.tile_pool(name="io", bufs=1) as io_pool,
            tc.tile_pool(name="xg", bufs=3) as xg_pool,
            tc.tile_pool(name="w", bufs=3) as w_pool,
            tc.tile_pool(name="act", bufs=3) as act_pool,
            tc.tile_pool(name="ps", bufs=4, space="PSUM") as ps_pool,
        ):
            # dummy activation to pull the ~2.7us ACT table load into the
            # DMA prologue instead of serializing before the first sigmoid
            warm = io_pool.tile([128, 1], f32)
            nc.vector.memset(warm[:], 0.0)
            nc.scalar.activation(out=warm[:], in_=warm[:], func=AF.Sigmoid)

            wsum_sb = io_pool.tile([H, NN], bf16)
            nc.sync.dma_start(out=wsum_sb[:], in_=wsum_h[:])
            bias_sb = io_pool.tile([H, NN, 3], f32)
            nc.sync.dma_start(out=bias_sb[:], in_=bias_h[:])
            g_sb = io_pool.tile([N, 4, 2, NN], f32)  # b = 2*dim1 + dim2

            # Software pipeline with a 1-node skew: in period n the PE runs
            # mm1(n) then mm2/X(n-1), so ScalarE's LUT stream for node n
            # overlaps node n-1's matmul/reduce tail.
            prev = None  # (n, hsc, wfct)

            def emit_tail_pe(state):
                m, hsc_m, wfct_m = state
                auxes = []
                for h in range(2):  # b-halves
                    aux = ps_pool.tile([128, 2, 512], f32, tag="ps")
                    auxes.append(aux)
                    for k in range(2):
                        kb = 2 * h + k
                        nc.tensor.matmul(
                            aux[:, k, 0:448],
                            lhsT=wfct_m[:],
                            rhs=hsc_m[:, 2 * kb : 2 * kb + 2, :],
                            start=True,
                            stop=True,
                        )
                # X transposed matvecs: lhsT = hsc chunk puts X on 112
                # partitions, so the psum->sbuf copy is FD=16 wide.
                for h in range(2):
                    for bb in range(4):
                        b = 4 * h + bb
                        for tch in range(2):
                            col = 448 + 2 * bb + tch
                            nc.tensor.matmul(
                                auxes[h][0:112, 0, col : col + 1],
                                lhsT=hsc_m[:, b, 112 * tch : 112 * (tch + 1)],
                                rhs=wsum_sb[:, m : m + 1],
                                start=True,
                                stop=True,
                            )
                return auxes, None

            def emit_tail_dve(state, auxes, xp):
                m = state[0]
                xt = act_pool.tile([128, 16], f32, tag="xt")
                for h in range(2):
                    nc.vector.reduce_sum(
                        g_sb[:, 2 * h : 2 * h + 2, :, m : m + 1],
                        auxes[h][:, :, 0:448].rearrange("p k (s t) -> p k s t", s=2),
                        axis=mybir.AxisListType.X,
                    )
                    nc.vector.tensor_copy(
                        xt[0:112, 8 * h : 8 * h + 8], auxes[h][0:112, 0, 448:456]
                    )
                nc.sync.dma_start(
                    out=bass.AP(xo_h, m * B * TAU, [[1, 112], [TAU, B], [112, 2]]),
                    in_=xt[0:112, :].rearrange("p (b t) -> p b t", t=2),
                )

            pend_late = None  # (n, c_sb, hsc2, wfct) -> needs tanh_c + hsc
            pend_tail = None  # (n, hsc, wfct) -> needs mm2/X/G-red

            for it in range(NN + 2):
                # (1) late stage of node it-1: tanh_c opens this ACT period
                tmp_tail = None
                if pend_late is not None:
                    m, c_m, hsc2_m, wfct_m = pend_late
                    tanhc = act_pool.tile([128, B, TAU], bf16, tag="tanhc")
                    nc.scalar.activation(out=tanhc[:], in_=c_m[:], func=AF.Tanh)
                    hsc = act_pool.tile([128, B, TAU], bf16, tag="hsc")
                    nc.vector.tensor_mul(hsc[:], hsc2_m[:], tanhc[:])
                    tmp_tail = (m, hsc, wfct_m)
                    pend_late = None

                # (2) tail stage of node it-2: mm2/X + reductions
                if pend_tail is not None:
                    auxes, xp = emit_tail_pe(pend_tail)
                    emit_tail_dve(pend_tail, auxes, xp)
                pend_tail = tmp_tail

                # (3) front stage of node it: mm1 + sigmoid/tanh LUTs
                if it < NN:
                    n = it
                    g0, r = divmod(n, 4)
                    if r == 0:
                        # x sliding windows for 4 nodes: partition 32r+l
                        # holds the node's b-concatenated x row shifted by l;
                        # each 256-wide b segment is then a shifted window
                        # (tail 224:256 is garbage, never read).
                        xg = xg_pool.tile([128, B, T], bf16, tag="xg")
                        for rr in range(4):
                            nc.sync.dma_start(
                                out=xg[32 * rr : 32 * rr + 32, :, :],
                                in_=bass.AP(
                                    x_h,
                                    (4 * g0 + rr) * B * T,
                                    [[1, L], [1, B * T]],
                                ).rearrange("l (b t) -> l b t", b=B),
                            )
                        w1t = w_pool.tile([128, 3, H], bf16, tag="w1t")
                        nc.sync.dma_start(out=w1t[:], in_=w1_h[g0])

                    wfct = w_pool.tile([H, N], bf16, tag="wfct")
                    nc.sync.dma_start(out=wfct[:], in_=wfc_h[n])
                    invb = act_pool.tile([128, B * TAU], bf16, tag="invb")
                    nc.sync.dma_start(
                        out=invb[:],
                        in_=bass.AP(invx_h, n * B * TAU, [[0, 128], [1, B * TAU]]),
                    )
                    invb = invb.rearrange("p (b t) -> p b t", b=B)

                    acts = []
                    for ci, func in ((0, AF.Sigmoid), (1, AF.Tanh), (2, AF.Sigmoid)):
                        a_sb = act_pool.tile([128, B, TAU], bf16, tag=f"act{ci}")
                        for h in range(2):  # b-halves -> 2-bank psum tiles
                            gp = ps_pool.tile([128, 2, 512], f32, tag="ps")
                            for k in range(2):
                                kb = 2 * h + k
                                nc.tensor.matmul(
                                    gp[:, k, 0:448],
                                    lhsT=w1t[32 * r : 32 * r + 32, ci, :],
                                    rhs=xg[
                                        32 * r : 32 * r + 32,
                                        2 * kb : 2 * kb + 2,
                                        0:TAU,
                                    ],
                                    start=True,
                                    stop=True,
                                    tile_position=(32 * r, 0),
                                )
                            nc.scalar.activation(
                                out=a_sb[:, 4 * h : 4 * h + 4, :].rearrange(
                                    "p (k s) t -> p k (s t)", k=2
                                ),
                                in_=gp[:, :, 0:448],
                                func=func,
                                bias=bias_sb[:, n, ci : ci + 1],
                                scale=1.0,
                            )
                        acts.append(a_sb)
                    sigi, tanhg, sigo = acts

                    c_sb = act_pool.tile([128, B, TAU], bf16, tag="c")
                    # split by b-half: c-h0 only needs the h0 LUT outputs, so
                    # it is ready ~2 ACT calls earlier and cannot be starved
                    # behind the previous node's G-reduce in the DVE FIFO
                    for h in range(2):
                        nc.vector.tensor_mul(
                            c_sb[:, 4 * h : 4 * h + 4, :],
                            sigi[:, 4 * h : 4 * h + 4, :],
                            tanhg[:, 4 * h : 4 * h + 4, :],
                        )
                    # hsc2 = sig_o * invx is independent of tanh_c, so only
                    # one DVE op (hsc) sits downstream of the tanh_c LUT.
                    hsc2 = act_pool.tile([128, B, TAU], bf16, tag="hsc2")
                    nc.vector.tensor_mul(hsc2[:], sigo[:], invb[:])

                if it < NN:
                    pend_late = (n, c_sb, hsc2, wfct)

            nc.sync.dma_start(
                out=g_h[:], in_=g_sb[:].rearrange("p k s n -> p (k s) n")
            )
    return nc


def get_nc():
    if "nc" not in _cache:
        nc = build_nc()
        nc.compile()
        _cache["nc"] = nc
    return _cache["nc"]


def make_in_maps(x, W_ih, b_ih, b_hh, W_fc, b_fc):
    x = np.asarray(x, np.float32)
    bias = np.asarray(b_ih, np.float32) + np.asarray(b_hh, np.float32)  # [N, 4H]
    W_ih = np.asarray(W_ih, np.float32)
    W_fc = np.asarray(W_fc, np.float32)

    xb = x.astype(BF16)  # [B, N, T]
    in_maps = []
    for core in range(NCORES):
        nd = slice(NN * core, NN * (core + 1))
        Wn = W_ih[nd]  # [NN, 4H, L]
        w1c = np.empty((NG, 128, 3, H), np.float32)
        for g0 in range(NG):
            for r in range(4):
                node = 4 * g0 + r
                for ci, cm in enumerate(CHUNKS):
                    w1c[g0, 32 * r : 32 * r + 32, ci, :] = Wn[
                        node, cm * H : (cm + 1) * H, :
                    ].T
        wfcc = np.ascontiguousarray(W_fc[nd].transpose(0, 2, 1))  # [NN, h, j]
        wsumc = np.ascontiguousarray(W_fc[nd].sum(axis=1).T)  # [h, NN]
        biasc = np.stack(
            [bias[nd][:, cm * H : (cm + 1) * H] for cm in CHUNKS], axis=-1
        )  # [NN, 128, 3]
        biasc = np.ascontiguousarray(biasc.transpose(1, 0, 2))  # [128, NN, 3]
        denom = x[:, nd, L:]  # [B, NN, TAU]
        invc = np.ascontiguousarray(
            (1.0 / (float(TAU) * denom)).transpose(1, 0, 2)
        )  # [NN, B, TAU]
        xc = np.ascontiguousarray(xb[:, nd, :].transpose(1, 0, 2)).reshape(-1)
        xc = np.concatenate([xc, np.zeros(L, BF16)])
        in_maps.append(
            {
                "x": xc,
                "w1": w1c.astype(BF16),
                "wfc": wfcc.astype(BF16),
                "wsum": wsumc.astype(BF16),
                "bias": np.ascontiguousarray(biasc, np.float32),
                "invx": invc.astype(BF16),
            }
        )
    return in_maps


def assemble(results, x, W_fc, b_fc):
    x = np.asarray(x, np.float32)
    W_fc = np.asarray(W_fc, np.float32)
    b_fc = np.asarray(b_fc, np.float32)

    G = np.empty((B, N, N), np.float32)
    X = np.empty((B, TAU, N), np.float32)
    for core in range(NCORES):
        nd = slice(NN * core, NN * (core + 1))
        g_dev = np.asarray(results[core]["gout"], np.float32)  # [N(j), B, NN]
        x_dev = np.asarray(results[core]["xout"], np.float32)  # [NN, B, TAU]
        G[:, :, nd] = g_dev.transpose(1, 0, 2)
        X[:, :, nd] = x_dev.transpose(1, 2, 0)

    invs = 1.0 / (float(TAU) * x[:, :, L:])  # [B, N, TAU]
    S = invs.sum(axis=2)  # [B, N] (i)
    G += b_fc.T[None, :, :] * S[:, None, :]
    X *= float(TAU) * x[:, :, L:].transpose(0, 2, 1)
    X += b_fc.sum(axis=1)[None, None, :] + 1e-6
    return G, X


def kernel(x, W_ih, b_ih, b_hh, W_fc, b_fc):
    nc = get_nc()
    in_maps = make_in_maps(x, W_ih, b_ih, b_hh, W_fc, b_fc)
    res = run_bass_kernel_spmd(nc, in_maps, core_ids=list(range(NCORES)))
    return assemble(res.results, x, W_fc, b_fc)


# revision 39
# speedup vs baseline: 1.0464x; 1.0464x over previous
"""Trainium2 Bass kernel for nn_BSI_71597104824914.

Model (per batch b, node i): single-step LSTM (zero init state) over sliding
windows of x, then a per-node FC producing contrib[b,tau,j,i]; outputs
  G[b,j,i]    = mean_tau contrib / x[b,i,L+tau]
  X_hat[b,tau,i] = sum_j contrib + 1e-6

Sharding: nodes are split across the 8 NeuronCores (16 nodes/core), each core
processes all 8 batch elements for its nodes. Per (node):
  mm1 (PE):   gates[g, b, tau] = sum_l W_ih[n,g,l] * x[b,n,tau+l]   (K=32)
  ACT:        sigmoid/tanh batched over (b, tau), bias via ACT bias port
  DVE:        c = sig_i*tanh_g; hsc = tanh(c)*sig_o * 1/(224*x)  (bf16, 2x)
  mm2 (PE):   contrib_scaled[j, b, tau] = WfcT_n.T @ hsc          (K=128)
  X (PE):     transposed matvec: lhsT=hsc chunk -> X on 112 partitions
  DVE:        G col = reduce_tau(contrib_scaled); tiny X psum->sbuf copy
Host folds: bias terms of G/X, un-scaling of X, gather/transpose.
"""

import numpy as np
import ml_dtypes

import concourse.bass as bass
import concourse.bacc as bacc
import concourse.tile as tile
from concourse.tile import add_dep_helper
from concourse import mybir
from concourse.bass_utils import run_bass_kernel_spmd

BF16 = ml_dtypes.bfloat16

B, N, T, L, H = 8, 128, 256, 32, 128
TAU = T - L  # 224
NCORES = 8
NN = N // NCORES  # 16 nodes per core
NG = NN // 4  # node groups of 4 (row-tile positions)
CHUNKS = (0, 2, 3)  # pytorch gate order i,f,g,o -> we need i,g,o (f unused)

_cache = {}


def build_nc():
    nc = bacc.Bacc(None, target_bir_lowering=False)
    f32 = mybir.dt.float32
    bf16 = mybir.dt.bfloat16
    AF = mybir.ActivationFunctionType

    # x is node-major [NN, B, T] flattened, padded by 32 so the shifted
    # window reads of the last node stay in bounds.
    x_h = nc.dram_tensor("x", [NN * B * T + L], bf16, kind="ExternalInput")
    w1_h = nc.dram_tensor("w1", [NG, 128, 3, H], bf16, kind="ExternalInput")
    wfc_h = nc.dram_tensor("wfc", [NN, H, N], bf16, kind="ExternalInput")
    wsum_h = nc.dram_tensor("wsum", [H, NN], bf16, kind="ExternalInput")
    bias_h = nc.dram_tensor("bias", [H, NN, 3], f32, kind="ExternalInput")
    invx_h = nc.dram_tensor("invx", [NN, B, TAU], bf16, kind="ExternalInput")
    g_h = nc.dram_tensor("gout", [N, B, NN], f32, kind="ExternalOutput")
    xo_h = nc.dram_tensor("xout", [NN, B, TAU], f32, kind="ExternalOutput")

    with tile.TileContext(nc) as tc:
        with (
            tc.tile_pool(name="io", bufs=1) as io_pool,
            tc.tile_pool(name="xg", bufs=3) as xg_pool,
            tc.tile_pool(name="w", bufs=3) as w_pool,
            tc.tile_pool(name="act", bufs=3) as act_pool,
            tc.tile_pool(name="ps", bufs=4, space="PSUM") as ps_pool,
        ):
            # dummy activation to pull the ~2.7us ACT table load into the
            # DMA prologue instead of serializing before the first sigmoid
            warm = io_pool.tile([128, 1], f32)
            nc.vector.memset(warm[:], 0.0)
            nc.scalar.activation(out=warm[:], in_=warm[:], func=AF.Sigmoid)

            wsum_sb = io_pool.tile([H, NN], bf16)
            nc.sync.dma_start(out=wsum_sb[:], in_=wsum_h[:])
            bias_sb = io_pool.tile([H, NN, 3], f32)
            nc.sync.dma_start(out=bias_sb[:], in_=bias_h[:])
            g_sb = io_pool.tile([N, 4, 2, NN], f32)  # b = 2*dim1 + dim2

            # Software pipeline with a 1-node skew: in period n the PE runs
            # mm1(n) then mm2/X(n-1), so ScalarE's LUT stream for node n
            # overlaps node n-1's matmul/reduce tail.
            prev = None  # (n, hsc, wfct)

            def emit_tail_pe(state):
                m, hsc_m, wfct_m = state
                auxes = []
                for h in range(2):  # b-halves
                    aux = ps_pool.tile([128, 2, 512], f32, tag="ps")
                    auxes.append(aux)
                    for k in range(2):
                        kb = 2 * h + k
                        nc.tensor.matmul(
                            aux[:, k, 0:448],
                            lhsT=wfct_m[:],
                            rhs=hsc_m[:, 2 * kb : 2 * kb + 2, :],
                            start=True,
                            stop=True,
                        )
                # X transposed matvecs: lhsT = hsc chunk puts X on 112
                # partitions, so the psum->sbuf copy is FD=16 wide.
                for h in range(2):
                    for bb in range(4):
                        b = 4 * h + bb
                        for tch in range(2):
                            col = 448 + 2 * bb + tch
                            nc.tensor.matmul(
                                auxes[h][0:112, 0, col : col + 1],
                                lhsT=hsc_m[:, b, 112 * tch : 112 * (tch + 1)],
                                rhs=wsum_sb[:, m : m + 1],
                                start=True,
                                stop=True,
                            )
                return auxes, None

            def emit_tail_dve(state, auxes, xp):
                m = state[0]
                xt = act_pool.tile([128, 16], f32, tag="xt")
                for h in range(2):
                    nc.vector.reduce_sum(
                        g_sb[:, 2 * h : 2 * h + 2, :, m : m + 1],
                        auxes[h][:, :, 0:448].rearrange("p k (s t) -> p k s t", s=2),
                        axis=mybir.AxisListType.X,
                    )
                    nc.vector.tensor_copy(
                        xt[0:112, 8 * h : 8 * h + 8], auxes[h][0:112, 0, 448:456]
                    )
                nc.sync.dma_start(
                    out=bass.AP(xo_h, m * B * TAU, [[1, 112], [TAU, B], [112, 2]]),
                    in_=xt[0:112, :].rearrange("p (b t) -> p b t", t=2),
                )

            pend_late = None  # (n, c_sb, hsc2, wfct) -> needs tanh_c + hsc
            pend_tail = None  # (n, hsc, wfct) -> needs mm2/X/G-red

            for it in range(NN + 2):
                # (1) late stage of node it-1: tanh_c opens this ACT period
                tmp_tail = None
                if pend_late is not None:
                    m, c_m, hsc2_m, wfct_m = pend_late
                    tanhc = act_pool.tile([128, B, TAU], bf16, tag="tanhc")
                    hsc = act_pool.tile([128, B, TAU], bf16, tag="hsc")
                    for h in range(2):
                        hs = slice(4 * h, 4 * h + 4)
                        nc.scalar.activation(
                            out=tanhc[:, hs, :], in_=c_m[:, hs, :], func=AF.Tanh
                        )
                        nc.vector.tensor_mul(
                            hsc[:, hs, :], hsc2_m[:, hs, :], tanhc[:, hs, :]
                        )
                    tmp_tail = (m, hsc, wfct_m)
                    pend_late = None

                # (2) tail stage of node it-2: mm2/X + reductions
                if pend_tail is not None:
                    auxes, xp = emit_tail_pe(pend_tail)
                    emit_tail_dve(pend_tail, auxes, xp)
                pend_tail = tmp_tail

                # (3) front stage of node it: mm1 + sigmoid/tanh LUTs
                if it < NN:
                    n = it
                    g0, r = divmod(n, 4)
                    if r == 0:
                        # x sliding windows for 4 nodes: partition 32r+l
                        # holds the node's b-concatenated x row shifted by l;
                        # each 256-wide b segment is then a shifted window
                        # (tail 224:256 is garbage, never read).
                        xg = xg_pool.tile([128, B, T], bf16, tag="xg")
                        for rr in range(4):
                            nc.sync.dma_start(
                                out=xg[32 * rr : 32 * rr + 32, :, :],
                                in_=bass.AP(
                                    x_h,
                                    (4 * g0 + rr) * B * T,
                                    [[1, L], [1, B * T]],
                                ).rearrange("l (b t) -> l b t", b=B),
                            )
                        w1t = w_pool.tile([128, 3, H], bf16, tag="w1t")
                        nc.sync.dma_start(out=w1t[:], in_=w1_h[g0])

                    wfct = w_pool.tile([H, N], bf16, tag="wfct")
                    nc.sync.dma_start(out=wfct[:], in_=wfc_h[n])
                    invb = act_pool.tile([128, B * TAU], bf16, tag="invb")
                    nc.sync.dma_start(
                        out=invb[:],
                        in_=bass.AP(invx_h, n * B * TAU, [[0, 128], [1, B * TAU]]),
                    )
                    invb = invb.rearrange("p (b t) -> p b t", b=B)

                    acts = []
                    for ci, func in ((0, AF.Sigmoid), (1, AF.Tanh), (2, AF.Sigmoid)):
                        a_sb = act_pool.tile([128, B, TAU], bf16, tag=f"act{ci}")
                        for h in range(2):  # b-halves -> 2-bank psum tiles
                            gp = ps_pool.tile([128, 2, 512], f32, tag="ps")
                            for k in range(2):
                                kb = 2 * h + k
                                nc.tensor.matmul(
                                    gp[:, k, 0:448],
                                    lhsT=w1t[32 * r : 32 * r + 32, ci, :],
                                    rhs=xg[
                                        32 * r : 32 * r + 32,
                                        2 * kb : 2 * kb + 2,
                                        0:TAU,
                                    ],
                                    start=True,
                                    stop=True,
                                    tile_position=(32 * r, 0),
                                )
                            nc.scalar.activation(
                                out=a_sb[:, 4 * h : 4 * h + 4, :].rearrange(
                                    "p (k s) t -> p k (s t)", k=2
                                ),
                                in_=gp[:, :, 0:448],
                                func=func,
                                bias=bias_sb[:, n, ci : ci + 1],
                                scale=1.0,
                            )
                        acts.append(a_sb)
                    sigi, tanhg, sigo = acts

                    c_sb = act_pool.tile([128, B, TAU], bf16, tag="c")
                    # split by b-half: c-h0 only needs the h0 LUT outputs, so
                    # it is ready ~2 ACT calls earlier and cannot be starved
                    # behind the previous node's G-reduce in the DVE FIFO
                    for h in range(2):
                        nc.vector.tensor_mul(
                            c_sb[:, 4 * h : 4 * h + 4, :],
                            sigi[:, 4 * h : 4 * h + 4, :],
                            tanhg[:, 4 * h : 4 * h + 4, :],
                        )
                    # hsc2 = sig_o * invx is independent of tanh_c, so only
                    # one DVE op (hsc) sits downstream of the tanh_c LUT.
                    hsc2 = act_pool.tile([128, B, TAU], bf16, tag="hsc2")
                    nc.vector.tensor_mul(hsc2[:], sigo[:], invb[:])

                if it < NN:
                    pend_late = (n, c_sb, hsc2, wfct)

            nc.sync.dma_start(
                out=g_h[:], in_=g_sb[:].rearrange("p k s n -> p (k s) n")
            )
    return nc


def get_nc():
    if "nc" not in _cache:
        nc = build_nc()
        nc.compile()
        _cache["nc"] = nc
    return _cache["nc"]


def make_in_maps(x, W_ih, b_ih, b_hh, W_fc, b_fc):
    x = np.asarray(x, np.float32)
    bias = np.asarray(b_ih, np.float32) + np.asarray(b_hh, np.float32)  # [N, 4H]
    W_ih = np.asarray(W_ih, np.float32)
    W_fc = np.asarray(W_fc, np.float32)

    xb = x.astype(BF16)  # [B, N, T]
    in_maps = []
    for core in range(NCORES):
        nd = slice(NN * core, NN * (core + 1))
        Wn = W_ih[nd]  # [NN, 4H, L]
        w1c = np.empty((NG, 128, 3, H), np.float32)
        for g0 in range(NG):
            for r in range(4):
                node = 4 * g0 + r
                for ci, cm in enumerate(CHUNKS):
                    w1c[g0, 32 * r : 32 * r + 32, ci, :] = Wn[
                        node, cm * H : (cm + 1) * H, :
                    ].T
        wfcc = np.ascontiguousarray(W_fc[nd].transpose(0, 2, 1))  # [NN, h, j]
        wsumc = np.ascontiguousarray(W_fc[nd].sum(axis=1).T)  # [h, NN]
        biasc = np.stack(
            [bias[nd][:, cm * H : (cm + 1) * H] for cm in CHUNKS], axis=-1
        )  # [NN, 128, 3]
        biasc = np.ascontiguousarray(biasc.transpose(1, 0, 2))  # [128, NN, 3]
        denom = x[:, nd, L:]  # [B, NN, TAU]
        invc = np.ascontiguousarray(
            (1.0 / (float(TAU) * denom)).transpose(1, 0, 2)
        )  # [NN, B, TAU]
        xc = np.ascontiguousarray(xb[:, nd, :].transpose(1, 0, 2)).reshape(-1)
        xc = np.concatenate([xc, np.zeros(L, BF16)])
        in_maps.append(
            {
                "x": xc,
                "w1": w1c.astype(BF16),
                "wfc": wfcc.astype(BF16),
                "wsum": wsumc.astype(BF16),
                "bias": np.ascontiguousarray(biasc, np.float32),
                "invx": invc.astype(BF16),
            }
        )
    return in_maps


def assemble(results, x, W_fc, b_fc):
    x = np.asarray(x, np.float32)
    W_fc = np.asarray(W_fc, np.float32)
    b_fc = np.asarray(b_fc, np.float32)

    G = np.empty((B, N, N), np.float32)
    X = np.empty((B, TAU, N), np.float32)
    for core in range(NCORES):
        nd = slice(NN * core, NN * (core + 1))
        g_dev = np.asarray(results[core]["gout"], np.float32)  # [N(j), B, NN]
        x_dev = np.asarray(results[core]["xout"], np.float32)  # [NN, B, TAU]
        G[:, :, nd] = g_dev.transpose(1, 0, 2)
        X[:, :, nd] = x_dev.transpose(1, 2, 0)

    invs = 1.0 / (float(TAU) * x[:, :, L:])  # [B, N, TAU]
    S = invs.sum(axis=2)  # [B, N] (i)
    G += b_fc.T[None, :, :] * S[:, None, :]
    X *= float(TAU) * x[:, :, L:].transpose(0, 2, 1)
    X += b_fc.sum(axis=1)[None, None, :] + 1e-6
    return G, X


def kernel(x, W_ih, b_ih, b_hh, W_fc, b_fc):
    nc = get_nc()
    in_maps = make_in_maps(x, W_ih, b_ih, b_hh, W_fc, b_fc)
    res = run_bass_kernel_spmd(nc, in_maps, core_ids=list(range(NCORES)))
    return assemble(res.results, x, W_fc, b_fc)


# revision 43
# speedup vs baseline: 1.1628x; 1.1113x over previous
"""Trainium2 Bass kernel for nn_BSI_71597104824914.

Model (per batch b, node i): single-step LSTM (zero init state) over sliding
windows of x, then a per-node FC producing contrib[b,tau,j,i]; outputs
  G[b,j,i]    = mean_tau contrib / x[b,i,L+tau]
  X_hat[b,tau,i] = sum_j contrib + 1e-6

Sharding: nodes are split across the 8 NeuronCores (16 nodes/core), each core
processes all 8 batch elements for its nodes. Per (node):
  mm1 (PE):   gates[g, b, tau] = sum_l W_ih[n,g,l] * x[b,n,tau+l]   (K=32)
  ACT:        sigmoid/tanh batched over (b, tau), bias via ACT bias port
  DVE:        c = sig_i*tanh_g; hsc = tanh(c)*sig_o * 1/(224*x)  (bf16, 2x)
  mm2 (PE):   contrib_scaled[j, b, tau] = WfcT_n.T @ hsc          (K=128)
  X (PE):     transposed matvec: lhsT=hsc chunk -> X on 112 partitions
  DVE:        G col = reduce_tau(contrib_scaled); tiny X psum->sbuf copy
Host folds: bias terms of G/X, un-scaling of X, gather/transpose.
"""

import numpy as np
import ml_dtypes

import concourse.bass as bass
import concourse.bacc as bacc
import concourse.tile as tile
from concourse.tile import add_dep_helper
from concourse import mybir
from concourse.bass_utils import run_bass_kernel_spmd

BF16 = ml_dtypes.bfloat16

B, N, T, L, H = 8, 128, 256, 32, 128
TAU = T - L  # 224
NCORES = 8
NN = N // NCORES  # 16 nodes per core
NG = NN // 4  # node groups of 4 (row-tile positions)
CHUNKS = (0, 2, 3)  # pytorch gate order i,f,g,o -> we need i,g,o (f unused)

_cache = {}


def build_nc():
    nc = bacc.Bacc(None, target_bir_lowering=False)
    f32 = mybir.dt.float32
    bf16 = mybir.dt.bfloat16
    AF = mybir.ActivationFunctionType

    # x is node-major [NN, B, T] flattened, padded by 32 so the shifted
    # window reads of the last node stay in bounds.
    x_h = nc.dram_tensor("x", [NN * B * T + L], bf16, kind="ExternalInput")
    w1_h = nc.dram_tensor("w1", [NG, 128, 3, H], bf16, kind="ExternalInput")
    wfc_h = nc.dram_tensor("wfc", [NN, H, N], bf16, kind="ExternalInput")
    wsum_h = nc.dram_tensor("wsum", [H, NN], bf16, kind="ExternalInput")
    wsel_h = nc.dram_tensor("wsel", [NN, H, B, B], bf16, kind="ExternalInput")
    bias_h = nc.dram_tensor("bias", [H, NN, 3], f32, kind="ExternalInput")
    invx_h = nc.dram_tensor("invx", [NN, B, TAU], bf16, kind="ExternalInput")
    g_h = nc.dram_tensor("gout", [N, B, NN], f32, kind="ExternalOutput")
    xo_h = nc.dram_tensor("xout", [NN, B, TAU], f32, kind="ExternalOutput")

    with tile.TileContext(nc) as tc:
        with (
            tc.tile_pool(name="io", bufs=1) as io_pool,
            tc.tile_pool(name="xg", bufs=3) as xg_pool,
            tc.tile_pool(name="w", bufs=3) as w_pool,
            tc.tile_pool(name="act", bufs=3) as act_pool,
            tc.tile_pool(name="ps", bufs=4, space="PSUM") as ps_pool,
        ):
            # dummy activation to pull the ~2.7us ACT table load into the
            # DMA prologue instead of serializing before the first sigmoid
            warm = io_pool.tile([128, 1], f32)
            nc.vector.memset(warm[:], 0.0)
            nc.scalar.activation(out=warm[:], in_=warm[:], func=AF.Sigmoid)

            wsum_sb = io_pool.tile([H, NN], bf16)
            nc.sync.dma_start(out=wsum_sb[:], in_=wsum_h[:])
            bias_sb = io_pool.tile([H, NN, 3], f32)
            nc.sync.dma_start(out=bias_sb[:], in_=bias_h[:])
            g_sb = io_pool.tile([N, 4, 2, NN], f32)  # b = 2*dim1 + dim2

            # Software pipeline with a 1-node skew: in period n the PE runs
            # mm1(n) then mm2/X(n-1), so ScalarE's LUT stream for node n
            # overlaps node n-1's matmul/reduce tail.
            prev = None  # (n, hsc, wfct)

            def emit_tail_pe(state):
                m, hsc_m, wfct_m, wsel_m = state
                auxes = []
                for h in range(2):  # b-halves
                    aux = ps_pool.tile([128, 2, 512], f32, tag="ps")
                    auxes.append(aux)
                    for k in range(2):
                        kb = 2 * h + k
                        nc.tensor.matmul(
                            aux[:, k, 0:448],
                            lhsT=wfct_m[:],
                            rhs=hsc_m[:, 2 * kb : 2 * kb + 2, :],
                            start=True,
                            stop=True,
                        )
                # X matvecs: stationary = wsum in column b, zeros elsewhere
                # (M=8, near-free weight load); the 8 matvecs ACCUMULATE into
                # one [8, 224] psum region, landing X rows on partitions 0-7.
                xp = ps_pool.tile([128, 2, 512], f32, tag="ps")
                for b in range(B):
                    nc.tensor.matmul(
                        xp[0:B, 0, 0:TAU],
                        lhsT=wsel_m[:, b, :],
                        rhs=hsc_m[:, b, :],
                        start=(b == 0),
                        stop=(b == B - 1),
                    )
                return auxes, xp

            def emit_tail_dve(state, auxes, xp):
                m = state[0]
                for h in range(2):
                    nc.vector.reduce_sum(
                        g_sb[:, 2 * h : 2 * h + 2, :, m : m + 1],
                        auxes[h][:, :, 0:448].rearrange("p k (s t) -> p k s t", s=2),
                        axis=mybir.AxisListType.X,
                    )
                xt = act_pool.tile([128, TAU], f32, tag="xt")
                nc.vector.tensor_copy(xt[0:B, :], xp[0:B, 0, 0:TAU])
                nc.sync.dma_start(
                    out=bass.AP(xo_h, m * B * TAU, [[TAU, B], [1, TAU]]),
                    in_=xt[0:B, :],
                )

            pend_late = None  # (n, c_sb, hsc2, wfct) -> needs tanh_c + hsc
            pend_tail = None  # (n, hsc, wfct) -> needs mm2/X/G-red

            for it in range(NN + 2):
                # (1) late stage of node it-1: tanh_c opens this ACT period
                tmp_tail = None
                if pend_late is not None:
                    m, c_m, hsc2_m, wfct_m, wsel_m = pend_late
                    tanhc = act_pool.tile([128, B, TAU], bf16, tag="tanhc")
                    hsc = act_pool.tile([128, B, TAU], bf16, tag="hsc")
                    for h in range(2):
                        hs = slice(4 * h, 4 * h + 4)
                        nc.scalar.activation(
                            out=tanhc[:, hs, :], in_=c_m[:, hs, :], func=AF.Tanh
                        )
                        nc.vector.tensor_mul(
                            hsc[:, hs, :], hsc2_m[:, hs, :], tanhc[:, hs, :]
                        )
                    tmp_tail = (m, hsc, wfct_m, wsel_m)
                    pend_late = None

                # (2) tail stage of node it-2: mm2/X + reductions
                if pend_tail is not None:
                    auxes, xp = emit_tail_pe(pend_tail)
                    emit_tail_dve(pend_tail, auxes, xp)
                pend_tail = tmp_tail

                # (3) front stage of node it: mm1 + sigmoid/tanh LUTs
                if it < NN:
                    n = it
                    g0, r = divmod(n, 4)
                    if r == 0:
                        # x sliding windows for 4 nodes: partition 32r+l
                        # holds the node's b-concatenated x row shifted by l;
                        # each 256-wide b segment is then a shifted window
                        # (tail 224:256 is garbage, never read).
                        xg = xg_pool.tile([128, B, T], bf16, tag="xg")
                        for rr in range(4):
                            nc.sync.dma_start(
                                out=xg[32 * rr : 32 * rr + 32, :, :],
                                in_=bass.AP(
                                    x_h,
                                    (4 * g0 + rr) * B * T,
                                    [[1, L], [1, B * T]],
                                ).rearrange("l (b t) -> l b t", b=B),
                            )
                        w1t = w_pool.tile([128, 3, H], bf16, tag="w1t")
                        nc.sync.dma_start(out=w1t[:], in_=w1_h[g0])

                    wfct = w_pool.tile([H, N], bf16, tag="wfct")
                    nc.sync.dma_start(out=wfct[:], in_=wfc_h[n])
                    wsel = w_pool.tile([H, B, B], bf16, tag="wsel")
                    nc.sync.dma_start(out=wsel[:], in_=wsel_h[n])
                    invb = act_pool.tile([128, B * TAU], bf16, tag="invb")
                    nc.sync.dma_start(
                        out=invb[:],
                        in_=bass.AP(invx_h, n * B * TAU, [[0, 128], [1, B * TAU]]),
                    )
                    invb = invb.rearrange("p (b t) -> p b t", b=B)

                    acts = []
                    for ci, func in ((0, AF.Sigmoid), (1, AF.Tanh), (2, AF.Sigmoid)):
                        a_sb = act_pool.tile([128, B, TAU], bf16, tag=f"act{ci}")
                        for h in range(2):  # b-halves -> 2-bank psum tiles
                            gp = ps_pool.tile([128, 2, 512], f32, tag="ps")
                            for k in range(2):
                                kb = 2 * h + k
                                nc.tensor.matmul(
                                    gp[:, k, 0:448],
                                    lhsT=w1t[32 * r : 32 * r + 32, ci, :],
                                    rhs=xg[
                                        32 * r : 32 * r + 32,
                                        2 * kb : 2 * kb + 2,
                                        0:TAU,
                                    ],
                                    start=True,
                                    stop=True,
                                    tile_position=(32 * r, 0),
                                )
                            nc.scalar.activation(
                                out=a_sb[:, 4 * h : 4 * h + 4, :].rearrange(
                                    "p (k s) t -> p k (s t)", k=2
                                ),
                                in_=gp[:, :, 0:448],
                                func=func,
                                bias=bias_sb[:, n, ci : ci + 1],
                                scale=1.0,
                            )
                        acts.append(a_sb)
                    sigi, tanhg, sigo = acts

                    c_sb = act_pool.tile([128, B, TAU], bf16, tag="c")
                    # split by b-half: c-h0 only needs the h0 LUT outputs, so
                    # it is ready ~2 ACT calls earlier and cannot be starved
                    # behind the previous node's G-reduce in the DVE FIFO
                    for h in range(2):
                        nc.vector.tensor_mul(
                            c_sb[:, 4 * h : 4 * h + 4, :],
                            sigi[:, 4 * h : 4 * h + 4, :],
                            tanhg[:, 4 * h : 4 * h + 4, :],
                        )
                    # hsc2 = sig_o * invx is independent of tanh_c, so only
                    # one DVE op (hsc) sits downstream of the tanh_c LUT.
                    hsc2 = act_pool.tile([128, B, TAU], bf16, tag="hsc2")
                    nc.vector.tensor_mul(hsc2[:], sigo[:], invb[:])

                if it < NN:
                    pend_late = (n, c_sb, hsc2, wfct, wsel)

            nc.sync.dma_start(
                out=g_h[:], in_=g_sb[:].rearrange("p k s n -> p (k s) n")
            )
    return nc


def get_nc():
    if "nc" not in _cache:
        nc = build_nc()
        nc.compile()
        _cache["nc"] = nc
    return _cache["nc"]


def make_in_maps(x, W_ih, b_ih, b_hh, W_fc, b_fc):
    x = np.asarray(x, np.float32)
    bias = np.asarray(b_ih, np.float32) + np.asarray(b_hh, np.float32)  # [N, 4H]
    W_ih = np.asarray(W_ih, np.float32)
    W_fc = np.asarray(W_fc, np.float32)

    xb = x.astype(BF16)  # [B, N, T]
    in_maps = []
    for core in range(NCORES):
        nd = slice(NN * core, NN * (core + 1))
        Wn = W_ih[nd]  # [NN, 4H, L]
        w1c = np.empty((NG, 128, 3, H), np.float32)
        for g0 in range(NG):
            for r in range(4):
                node = 4 * g0 + r
                for ci, cm in enumerate(CHUNKS):
                    w1c[g0, 32 * r : 32 * r + 32, ci, :] = Wn[
                        node, cm * H : (cm + 1) * H, :
                    ].T
        wfcc = np.ascontiguousarray(W_fc[nd].transpose(0, 2, 1))  # [NN, h, j]
        wsumc = np.ascontiguousarray(W_fc[nd].sum(axis=1).T)  # [h, NN]
        wselc = np.zeros((NN, H, B, B), np.float32)  # [n, h, b, col]; col b = wsum
        for b in range(B):
            wselc[:, :, b, b] = W_fc[nd].sum(axis=1)
        biasc = np.stack(
            [bias[nd][:, cm * H : (cm + 1) * H] for cm in CHUNKS], axis=-1
        )  # [NN, 128, 3]
        biasc = np.ascontiguousarray(biasc.transpose(1, 0, 2))  # [128, NN, 3]
        denom = x[:, nd, L:]  # [B, NN, TAU]
        invc = np.ascontiguousarray(
            (1.0 / (float(TAU) * denom)).transpose(1, 0, 2)
        )  # [NN, B, TAU]
        xc = np.ascontiguousarray(xb[:, nd, :].transpose(1, 0, 2)).reshape(-1)
        xc = np.concatenate([xc, np.zeros(L, BF16)])
        in_maps.append(
            {
                "x": xc,
                "w1": w1c.astype(BF16),
                "wfc": wfcc.astype(BF16),
                "wsum": wsumc.astype(BF16),
                "wsel": wselc.astype(BF16),
                "bias": np.ascontiguousarray(biasc, np.float32),
                "invx": invc.astype(BF16),
            }
        )
    return in_maps


def assemble(results, x, W_fc, b_fc):
    x = np.asarray(x, np.float32)
    W_fc = np.asarray(W_fc, np.float32)
    b_fc = np.asarray(b_fc, np.float32)

    G = np.empty((B, N, N), np.float32)
    X = np.empty((B, TAU, N), np.float32)
    for core in range(NCORES):
        nd = slice(NN * core, NN * (core + 1))
        g_dev = np.asarray(results[core]["gout"], np.float32)  # [N(j), B, NN]
        x_dev = np.asarray(results[core]["xout"], np.float32)  # [NN, B, TAU]
        G[:, :, nd] = g_dev.transpose(1, 0, 2)
        X[:, :, nd] = x_dev.transpose(1, 2, 0)

    invs = 1.0 / (float(TAU) * x[:, :, L:])  # [B, N, TAU]
    S = invs.sum(axis=2)  # [B, N] (i)
    G += b_fc.T[None, :, :] * S[:, None, :]
    X *= float(TAU) * x[:, :, L:].transpose(0, 2, 1)
    X += b_fc.sum(axis=1)[None, None, :] + 1e-6
    return G, X


def kernel(x, W_ih, b_ih, b_hh, W_fc, b_fc):
    nc = get_nc()
    in_maps = make_in_maps(x, W_ih, b_ih, b_hh, W_fc, b_fc)
    res = run_bass_kernel_spmd(nc, in_maps, core_ids=list(range(NCORES)))
    return assemble(res.results, x, W_fc, b_fc)


# revision 44
# speedup vs baseline: 1.1683x; 1.0047x over previous
"""Trainium2 Bass kernel for nn_BSI_71597104824914.

Model (per batch b, node i): single-step LSTM (zero init state) over sliding
windows of x, then a per-node FC producing contrib[b,tau,j,i]; outputs
  G[b,j,i]    = mean_tau contrib / x[b,i,L+tau]
  X_hat[b,tau,i] = sum_j contrib + 1e-6

Sharding: nodes are split across the 8 NeuronCores (16 nodes/core), each core
processes all 8 batch elements for its nodes. Per (node):
  mm1 (PE):   gates[g, b, tau] = sum_l W_ih[n,g,l] * x[b,n,tau+l]   (K=32)
  ACT:        sigmoid/tanh batched over (b, tau), bias via ACT bias port
  DVE:        c = sig_i*tanh_g; hsc = tanh(c)*sig_o * 1/(224*x)  (bf16, 2x)
  mm2 (PE):   contrib_scaled[j, b, tau] = WfcT_n.T @ hsc          (K=128)
  X (PE):     transposed matvec: lhsT=hsc chunk -> X on 112 partitions
  DVE:        G col = reduce_tau(contrib_scaled); tiny X psum->sbuf copy
Host folds: bias terms of G/X, un-scaling of X, gather/transpose.
"""

import numpy as np
import ml_dtypes

import concourse.bass as bass
import concourse.bacc as bacc
import concourse.tile as tile
from concourse.tile import add_dep_helper
from concourse import mybir
from concourse.bass_utils import run_bass_kernel_spmd

BF16 = ml_dtypes.bfloat16

B, N, T, L, H = 8, 128, 256, 32, 128
TAU = T - L  # 224
NCORES = 8
NN = N // NCORES  # 16 nodes per core
NG = NN // 4  # node groups of 4 (row-tile positions)
CHUNKS = (0, 2, 3)  # pytorch gate order i,f,g,o -> we need i,g,o (f unused)

_cache = {}


def build_nc():
    nc = bacc.Bacc(None, target_bir_lowering=False)
    f32 = mybir.dt.float32
    bf16 = mybir.dt.bfloat16
    AF = mybir.ActivationFunctionType

    # x is node-major [NN, B, T] flattened, padded by 32 so the shifted
    # window reads of the last node stay in bounds.
    x_h = nc.dram_tensor("x", [NN * B * T + L], bf16, kind="ExternalInput")
    w1_h = nc.dram_tensor("w1", [NG, 128, 3, H], bf16, kind="ExternalInput")
    wfc_h = nc.dram_tensor("wfc", [NN, H, N], bf16, kind="ExternalInput")
    wsum_h = nc.dram_tensor("wsum", [H, NN], bf16, kind="ExternalInput")
    wsel_h = nc.dram_tensor("wsel", [NN, H, B, B], bf16, kind="ExternalInput")
    bias_h = nc.dram_tensor("bias", [H, NN, 3], f32, kind="ExternalInput")
    invx_h = nc.dram_tensor("invx", [NN, B, TAU], bf16, kind="ExternalInput")
    g_h = nc.dram_tensor("gout", [N, B, NN], f32, kind="ExternalOutput")
    xo_h = nc.dram_tensor("xout", [NN, B, TAU], f32, kind="ExternalOutput")

    with tile.TileContext(nc) as tc:
        with (
            tc.tile_pool(name="io", bufs=1) as io_pool,
            tc.tile_pool(name="xg", bufs=3) as xg_pool,
            tc.tile_pool(name="w", bufs=3) as w_pool,
            tc.tile_pool(name="act", bufs=3) as act_pool,
            tc.tile_pool(name="ps", bufs=4, space="PSUM") as ps_pool,
        ):
            # dummy activation to pull the ~2.7us ACT table load into the
            # DMA prologue instead of serializing before the first sigmoid
            warm = io_pool.tile([128, 1], f32)
            nc.vector.memset(warm[:], 0.0)
            nc.scalar.activation(out=warm[:], in_=warm[:], func=AF.Sigmoid)

            wsum_sb = io_pool.tile([H, NN], bf16)
            nc.sync.dma_start(out=wsum_sb[:], in_=wsum_h[:])
            bias_sb = io_pool.tile([H, NN, 3], f32)
            nc.sync.dma_start(out=bias_sb[:], in_=bias_h[:])
            g_sb = io_pool.tile([N, 4, 2, NN], f32)  # b = 2*dim1 + dim2

            # Software pipeline with a 1-node skew: in period n the PE runs
            # mm1(n) then mm2/X(n-1), so ScalarE's LUT stream for node n
            # overlaps node n-1's matmul/reduce tail.
            prev = None  # (n, hsc, wfct)

            def emit_tail_pe(state):
                m, hsc_m, wfct_m, wsel_m = state
                auxes = []
                for h in range(2):  # b-halves
                    aux = ps_pool.tile([128, 2, 512], f32, tag="ps")
                    auxes.append(aux)
                    for k in range(2):
                        kb = 2 * h + k
                        nc.tensor.matmul(
                            aux[:, k, 0:448],
                            lhsT=wfct_m[:],
                            rhs=hsc_m[:, 2 * kb : 2 * kb + 2, :],
                            start=True,
                            stop=True,
                        )
                # X matvecs: stationary = wsum in column b, zeros elsewhere
                # (M=8, near-free weight load); the 8 matvecs ACCUMULATE into
                # one [8, 224] psum region, landing X rows on partitions 0-7.
                xp = ps_pool.tile([128, 2, 512], f32, tag="ps")
                for b in range(B):
                    nc.tensor.matmul(
                        xp[0:B, 0, 0:TAU],
                        lhsT=wsel_m[:, b, :],
                        rhs=hsc_m[:, b, :],
                        start=(b == 0),
                        stop=(b == B - 1),
                    )
                return auxes, xp

            def emit_tail_dve(state, auxes, xp):
                m = state[0]
                for h in range(2):
                    nc.vector.reduce_sum(
                        g_sb[:, 2 * h : 2 * h + 2, :, m : m + 1],
                        auxes[h][:, :, 0:448].rearrange("p k (s t) -> p k s t", s=2),
                        axis=mybir.AxisListType.X,
                    )
                xt = act_pool.tile([128, TAU], f32, tag="xt")
                nc.vector.tensor_copy(xt[0:B, :], xp[0:B, 0, 0:TAU])
                nc.sync.dma_start(
                    out=bass.AP(xo_h, m * B * TAU, [[TAU, B], [1, TAU]]),
                    in_=xt[0:B, :],
                )

            pend_late = None  # (n, c_sb, hsc2, wfct) -> needs tanh_c + hsc
            pend_tail = None  # (n, hsc, wfct) -> needs mm2/X/G-red

            for it in range(NN + 2):
                # (1) late stage of node it-1: tanh_c opens this ACT period
                tmp_tail = None
                if pend_late is not None:
                    m, c_m, hsc2_m, wfct_m, wsel_m = pend_late
                    tanhc = act_pool.tile([128, B, TAU], bf16, tag="tanhc")
                    hsc = act_pool.tile([128, B, TAU], bf16, tag="hsc")
                    nc.scalar.activation(out=tanhc[:], in_=c_m[:], func=AF.Tanh)
                    nc.vector.tensor_mul(hsc[:], hsc2_m[:], tanhc[:])
                    tmp_tail = (m, hsc, wfct_m, wsel_m)
                    pend_late = None

                # (2) tail stage of node it-2: mm2/X + reductions
                if pend_tail is not None:
                    auxes, xp = emit_tail_pe(pend_tail)
                    emit_tail_dve(pend_tail, auxes, xp)
                pend_tail = tmp_tail

                # (3) front stage of node it: mm1 + sigmoid/tanh LUTs
                if it < NN:
                    n = it
                    g0, r = divmod(n, 4)
                    if r == 0:
                        # x sliding windows for 4 nodes: partition 32r+l
                        # holds the node's b-concatenated x row shifted by l;
                        # each 256-wide b segment is then a shifted window
                        # (tail 224:256 is garbage, never read).
                        xg = xg_pool.tile([128, B, T], bf16, tag="xg")
                        for rr in range(4):
                            nc.sync.dma_start(
                                out=xg[32 * rr : 32 * rr + 32, :, :],
                                in_=bass.AP(
                                    x_h,
                                    (4 * g0 + rr) * B * T,
                                    [[1, L], [1, B * T]],
                                ).rearrange("l (b t) -> l b t", b=B),
                            )
                        w1t = w_pool.tile([128, 3, H], bf16, tag="w1t")
                        nc.sync.dma_start(out=w1t[:], in_=w1_h[g0])

                    wfct = w_pool.tile([H, N], bf16, tag="wfct")
                    nc.sync.dma_start(out=wfct[:], in_=wfc_h[n])
                    wsel = w_pool.tile([H, B, B], bf16, tag="wsel")
                    nc.sync.dma_start(out=wsel[:], in_=wsel_h[n])
                    invb = act_pool.tile([128, B * TAU], bf16, tag="invb")
                    nc.sync.dma_start(
                        out=invb[:],
                        in_=bass.AP(invx_h, n * B * TAU, [[0, 128], [1, B * TAU]]),
                    )
                    invb = invb.rearrange("p (b t) -> p b t", b=B)

                    acts = []
                    for ci, func in ((0, AF.Sigmoid), (1, AF.Tanh), (2, AF.Sigmoid)):
                        a_sb = act_pool.tile([128, B, TAU], bf16, tag=f"act{ci}")
                        for h in range(2):  # b-halves -> 2-bank psum tiles
                            gp = ps_pool.tile([128, 2, 512], f32, tag="ps")
                            for k in range(2):
                                kb = 2 * h + k
                                nc.tensor.matmul(
                                    gp[:, k, 0:448],
                                    lhsT=w1t[32 * r : 32 * r + 32, ci, :],
                                    rhs=xg[
                                        32 * r : 32 * r + 32,
                                        2 * kb : 2 * kb + 2,
                                        0:TAU,
                                    ],
                                    start=True,
                                    stop=True,
                                    tile_position=(32 * r, 0),
                                )
                            nc.scalar.activation(
                                out=a_sb[:, 4 * h : 4 * h + 4, :].rearrange(
                                    "p (k s) t -> p k (s t)", k=2
                                ),
                                in_=gp[:, :, 0:448],
                                func=func,
                                bias=bias_sb[:, n, ci : ci + 1],
                                scale=1.0,
                            )
                        acts.append(a_sb)
                    sigi, tanhg, sigo = acts

                    c_sb = act_pool.tile([128, B, TAU], bf16, tag="c")
                    # split by b-half: c-h0 only needs the h0 LUT outputs, so
                    # it is ready ~2 ACT calls earlier and cannot be starved
                    # behind the previous node's G-reduce in the DVE FIFO
                    for h in range(2):
                        nc.vector.tensor_mul(
                            c_sb[:, 4 * h : 4 * h + 4, :],
                            sigi[:, 4 * h : 4 * h + 4, :],
                            tanhg[:, 4 * h : 4 * h + 4, :],
                        )
                    # hsc2 = sig_o * invx is independent of tanh_c, so only
                    # one DVE op (hsc) sits downstream of the tanh_c LUT.
                    hsc2 = act_pool.tile([128, B, TAU], bf16, tag="hsc2")
                    nc.vector.tensor_mul(hsc2[:], sigo[:], invb[:])

                if it < NN:
                    pend_late = (n, c_sb, hsc2, wfct, wsel)

            nc.sync.dma_start(
                out=g_h[:], in_=g_sb[:].rearrange("p k s n -> p (k s) n")
            )
    return nc


def get_nc():
    if "nc" not in _cache:
        nc = build_nc()
        nc.compile()
        _cache["nc"] = nc
    return _cache["nc"]


def make_in_maps(x, W_ih, b_ih, b_hh, W_fc, b_fc):
    x = np.asarray(x, np.float32)
    bias = np.asarray(b_ih, np.float32) + np.asarray(b_hh, np.float32)  # [N, 4H]
    W_ih = np.asarray(W_ih, np.float32)
    W_fc = np.asarray(W_fc, np.float32)

    xb = x.astype(BF16)  # [B, N, T]
    in_maps = []
    for core in range(NCORES):
        nd = slice(NN * core, NN * (core + 1))
        Wn = W_ih[nd]  # [NN, 4H, L]
        w1c = np.empty((NG, 128, 3, H), np.float32)
        for g0 in range(NG):
            for r in range(4):
                node = 4 * g0 + r
                for ci, cm in enumerate(CHUNKS):
                    w1c[g0, 32 * r : 32 * r + 32, ci, :] = Wn[
                        node, cm * H : (cm + 1) * H, :
                    ].T
        wfcc = np.ascontiguousarray(W_fc[nd].transpose(0, 2, 1))  # [NN, h, j]
        wsumc = np.ascontiguousarray(W_fc[nd].sum(axis=1).T)  # [h, NN]
        wselc = np.zeros((NN, H, B, B), np.float32)  # [n, h, b, col]; col b = wsum
        for b in range(B):
            wselc[:, :, b, b] = W_fc[nd].sum(axis=1)
        biasc = np.stack(
            [bias[nd][:, cm * H : (cm + 1) * H] for cm in CHUNKS], axis=-1
        )  # [NN, 128, 3]
        biasc = np.ascontiguousarray(biasc.transpose(1, 0, 2))  # [128, NN, 3]
        denom = x[:, nd, L:]  # [B, NN, TAU]
        invc = np.ascontiguousarray(
            (1.0 / (float(TAU) * denom)).transpose(1, 0, 2)
        )  # [NN, B, TAU]
        xc = np.ascontiguousarray(xb[:, nd, :].transpose(1, 0, 2)).reshape(-1)
        xc = np.concatenate([xc, np.zeros(L, BF16)])
        in_maps.append(
            {
                "x": xc,
                "w1": w1c.astype(BF16),
                "wfc": wfcc.astype(BF16),
                "wsum": wsumc.astype(BF16),
                "wsel": wselc.astype(BF16),
                "bias": np.ascontiguousarray(biasc, np.float32),
                "invx": invc.astype(BF16),
            }
        )
    return in_maps


def assemble(results, x, W_fc, b_fc):
    x = np.asarray(x, np.float32)
    W_fc = np.asarray(W_fc, np.float32)
    b_fc = np.asarray(b_fc, np.float32)

    G = np.empty((B, N, N), np.float32)
    X = np.empty((B, TAU, N), np.float32)
    for core in range(NCORES):
        nd = slice(NN * core, NN * (core + 1))
        g_dev = np.asarray(results[core]["gout"], np.float32)  # [N(j), B, NN]
        x_dev = np.asarray(results[core]["xout"], np.float32)  # [NN, B, TAU]
        G[:, :, nd] = g_dev.transpose(1, 0, 2)
        X[:, :, nd] = x_dev.transpose(1, 2, 0)

    invs = 1.0 / (float(TAU) * x[:, :, L:])  # [B, N, TAU]
    S = invs.sum(axis=2)  # [B, N] (i)
    G += b_fc.T[None, :, :] * S[:, None, :]
    X *= float(TAU) * x[:, :, L:].transpose(0, 2, 1)
    X += b_fc.sum(axis=1)[None, None, :] + 1e-6
    return G, X


def kernel(x, W_ih, b_ih, b_hh, W_fc, b_fc):
    nc = get_nc()
    in_maps = make_in_maps(x, W_ih, b_ih, b_hh, W_fc, b_fc)
    res = run_bass_kernel_spmd(nc, in_maps, core_ids=list(range(NCORES)))
    return assemble(res.results, x, W_fc, b_fc)
